# revision 28
# baseline (speedup 1.0000x reference)
"""Trainium2 Bass kernel for nn_PlasticityModelMoE (8-core SPMD), v2.

Strategy:
  Units tensor-parallel phase 1 (256 units/core): host supplies xT (bf16)
  and b-major w/delay (bf16); wmod = w*sigmoid(delay) on device. Per
  128-row batch tile, dk-outer matmul loop shares each xT stationary
  across the two 512-col branch groups + the 4-col gate group. Gate
  softmax (ACT Exp with accum_out, 1/sum folded into the relu's scale),
  z weighted sum, conn*mask, ReLU, degree-4 Horner blend in bf16, PE
  transposes (emitted one tile late so they never stall on the DVE
  chain).
  Schedule: tiny warmup collective at t=0 absorbs the ~40-80us CC-stream
  bring-up + inter-core launch skew; all four phase-1 chunks run first
  (their AllGather triggers all precede any ReduceScatter trigger on the
  in-order collective queue); then per chunk: phase 3 (memory-rows TP,
  1024 rows/core): logitsT = read_W x blendT, expT = exp(logitsT +
  read_b); phase 4: [read | s] = expT @ [mem | 1] with mk-outer shared
  stationaries (tiny s-column matmul first so the next weight load hides
  under a 512-col stream); bf16 ReduceScatter over batch rows (whole-
  chunk for ch0-2, halves for ch3 so the tail only exposes one half);
  all epilogues (divide by s, store 32/64-row slices) run at the end.
"""
import numpy as np
import ml_dtypes
from contextlib import ExitStack

import concourse.bass as bass
import concourse.mybir as mybir
import concourse.tile as tile
from concourse import bacc
from concourse.bass_utils import run_bass_kernel_spmd
from concourse.masks import make_identity

F32 = mybir.dt.float32
BF16 = mybir.dt.bfloat16
AF = mybir.ActivationFunctionType
ALU = mybir.AluOpType
AX = mybir.AxisListType

KC = 8
N, D, U, NB, M, MD = 2048, 1024, 2048, 4, 8192, 1024
US = U // KC          # 256 units per core
MS = M // KC          # 1024 memory rows per core
NS = N // KC          # 256 output rows per core
DK = D // 128         # 8 k-tiles over D
UK = U // 128         # 16 k-tiles over U
MK = MS // 128        # 8 k-tiles over memory shard
UBF = US * NB         # 1024 branch columns per core
NCH = 4               # batch chunks
CW = N // NCH         # 512 columns per chunk
RS_GROUPS = [[4], [4], [4], [2, 2]]  # sj per ReduceScatter, per chunk

_CMAT = np.array([
    [5.0000238e-01, 2.4987496e-01, 1.0582031e-03, -2.4046743e-02, 4.1678566e-03],
    [0.0, 1.0, 0.0, 0.0, 0.0],
    [-7.2632770e-06, 9.9976927e-01, 9.2018498e-03, -3.9401752e-01, 1.4669961e-01],
    [0.0, 1.0, 0.0, 0.0, 0.0],
    [8.6798245e-06, 4.9957812e-01, 2.5321743e-01, -8.1970906e-03, -1.3558048e-02],
    [3.9388153e-05, 4.9807969e-01, 4.1364601e-01, -3.7666172e-02, -3.2796454e-02],
    [0.0, 1.0507009873554805, 0.0, 0.0, 0.0],
    [3.1482985e-05, 5.9846270e-01, 3.3178753e-01, -4.6201140e-02, -1.9015398e-02],
    [0.0, 0.0, 0.0, 0.0, 0.0],
], dtype=np.float32)

_cache = {}


def _build(with_bias):
    nc = bacc.Bacc(num_devices=KC)

    xt_d = nc.dram_tensor("xt", [D, N], BF16, kind="ExternalInput")
    wd_d = nc.dram_tensor("wd", [D, UBF + NB], BF16, kind="ExternalInput")
    dd_d = nc.dram_tensor("dd", [D, UBF], BF16, kind="ExternalInput")
    bias_d = nc.dram_tensor("bias", [UBF + NB], F32, kind="ExternalInput")
    na_d = nc.dram_tensor("na", [U], F32, kind="ExternalInput")
    cw1_d = nc.dram_tensor("cw1", [U, 32], F32, kind="ExternalInput")
    cb1_d = nc.dram_tensor("cb1", [32], F32, kind="ExternalInput")
    cw2_d = nc.dram_tensor("cw2", [32, US], F32, kind="ExternalInput")
    cb2_d = nc.dram_tensor("cb2", [US], F32, kind="ExternalInput")
    mask_d = nc.dram_tensor("maskv", [US], F32, kind="ExternalInput")
    actw_d = nc.dram_tensor("actw", [9], F32, kind="ExternalInput")
    rw_d = nc.dram_tensor("rw", [U, MS], BF16, kind="ExternalInput")
    rb_d = nc.dram_tensor("rb", [MS], F32, kind="ExternalInput")
    mem_d = nc.dram_tensor("mem", [MS, MD], BF16, kind="ExternalInput")
    cmat_d = nc.dram_tensor("cmat", [9, 5], F32, kind="ExternalInput")
    y_d = nc.dram_tensor("y", [NS, MD], F32, kind="ExternalOutput")

    with tile.TileContext(nc) as tc, ExitStack() as ctx:
        consts = ctx.enter_context(tc.tile_pool(name="consts", bufs=1))
        big = ctx.enter_context(tc.tile_pool(name="big", bufs=1))
        st = ctx.enter_context(tc.tile_pool(name="st", bufs=2))
        blendp = ctx.enter_context(tc.tile_pool(name="blendp", bufs=2))
        p34 = ctx.enter_context(tc.tile_pool(name="p34", bufs=2))
        dram_cc = ctx.enter_context(tc.tile_pool(name="dram_cc", bufs=1,
                                                 space="DRAM"))
        # PSUM budget (8 banks, bank-granular per tag-buf):
        #   br [128,1024] f32 = 2 banks x2 = 4  (ph1 branch; ph4 read)
        #   l  [128,512]  f32 = 1 bank  x2 = 2  (setup misc; ph3 logits)
        #   sm [128,<=128]    = 1 bank  x2 = 2  (transposes, gate, s col)
        psum = ctx.enter_context(tc.tile_pool(name="psum", bufs=2, space="PSUM"))

        # -------- tiny setup DMAs first so setup PE work starts at t~0 -----
        aw = consts.tile([1, 9], F32)
        nc.sync.dma_start(out=aw, in_=actw_d.ap()[None])
        cmat_sb = consts.tile([9, 5], F32)
        nc.sync.dma_start(out=cmat_sb, in_=cmat_d[:, :])
        na_sb = consts.tile([128, UK], F32)
        nc.sync.dma_start(out=na_sb, in_=na_d.ap().rearrange("(t p) -> p t", p=128))
        cw1_sb = consts.tile([128, UK, 32], F32)
        nc.sync.dma_start(out=cw1_sb,
                          in_=cw1_d.ap().rearrange("(t p) c -> p t c", p=128))
        cb1_sb = consts.tile([1, 32], F32)
        nc.sync.dma_start(out=cb1_sb, in_=cb1_d.ap()[None])
        cw2_sb = consts.tile([32, US], F32)
        nc.sync.dma_start(out=cw2_sb, in_=cw2_d[:, :])
        cb2_sb = consts.tile([1, US], F32)
        nc.sync.dma_start(out=cb2_sb, in_=cb2_d.ap()[None])
        mask_sb = consts.tile([1, US], F32)
        nc.sync.dma_start(out=mask_sb, in_=mask_d.ap()[None])
        rb_sb = consts.tile([128, MK], F32)
        nc.sync.dma_start(out=rb_sb, in_=rb_d.ap().rearrange("(t p) -> p t", p=128))

        # tiny warmup collective: absorbs the ~40us first-collective cost
        # (CC stream bring-up + inter-core launch skew) off the AG0 path
        warm_pool = ctx.enter_context(tc.tile_pool(name="warm_pool", bufs=1,
                                                   space="DRAM"))
        warm_sb = consts.tile([1, 16], BF16)
        nc.vector.memset(warm_sb, 0.0)
        warm_in = warm_pool.tile([1, 16], BF16, name="warm_in", tag="wi")
        nc.gpsimd.dma_start(out=warm_in, in_=warm_sb)
        warm_out = warm_pool.tile([KC, 16], BF16, name="warm_out", tag="wo",
                                  addr_space="Shared")
        nc.gpsimd.collective_compute(
            "AllGather", ALU.bypass, replica_groups=[list(range(KC))],
            ins=[warm_in.opt()], outs=[warm_out.opt()],
        )

        # ---------------- startup DMAs: wd/dd/xt(ch0) on sync queue --------
        wd_s = []
        dd_s = []
        xt_c = [None] * NCH
        xt_c[0] = st.tile([128, DK, CW], BF16, tag="xt", name="xt_c0")
        for dk in range(DK):
            w_b = st.tile([128, UBF + NB], BF16, tag="lw", name=f"w_b{dk}")
            nc.sync.dma_start(out=w_b, in_=wd_d[dk * 128:(dk + 1) * 128, :])
            d_b = st.tile([128, UBF], BF16, tag="ld", name=f"d_b{dk}")
            nc.sync.dma_start(out=d_b, in_=dd_d[dk * 128:(dk + 1) * 128, :])
            nc.sync.dma_start(out=xt_c[0][:, dk, :],
                              in_=xt_d[dk * 128:(dk + 1) * 128, 0:CW])
            wd_s.append(w_b)
            dd_s.append(d_b)

        rw_sb = big.tile([128, UK, MS], BF16)
        mem_sb = big.tile([128, MK, MD + 1], BF16)

        # ---------------- wmod = w * sigmoid(delay), bf16 b-major ----------
        wmod_sb = big.tile([128, DK, UBF + NB], BF16)
        for dk in range(DK):
            sig_b = st.tile([128, UBF], BF16, tag="sg", name=f"sig{dk}")
            nc.scalar.activation(sig_b, dd_s[dk], AF.Sigmoid)
            nc.vector.tensor_mul(wmod_sb[:, dk, 0:UBF], wd_s[dk][:, 0:UBF],
                                 sig_b)
            nc.any.tensor_copy(wmod_sb[:, dk, UBF:UBF + NB],
                               wd_s[dk][:, UBF:UBF + NB])

        # ---------------- Setup A: tiny constants ----------------
        idf = consts.tile([128, 128], F32)
        make_identity(nc, idf)
        idb = consts.tile([128, 128], BF16)
        nc.any.tensor_copy(idb, idf)
        ones_lhs = consts.tile([1, 128], BF16)
        nc.vector.memset(ones_lhs, 1.0)
        ones_f = consts.tile([1, 128], F32)
        nc.vector.memset(ones_f, 1.0)
        idf1 = consts.tile([1, 1], F32)
        nc.vector.memset(idf1, 1.0)

        # softmax(act_w); polynomial coefs = wts @ cmat, broadcast to [128, 5]
        aw_negmax = consts.tile([1, 1], F32)
        nc.vector.tensor_reduce(aw_negmax, aw, AX.X, ALU.max, negate=True)
        aw_exp = consts.tile([1, 9], F32)
        aw_sum = consts.tile([1, 1], F32)
        nc.scalar.activation(aw_exp, aw, AF.Exp, bias=aw_negmax,
                             accum_out=aw_sum)
        aw_rec = consts.tile([1, 1], F32)
        nc.vector.reciprocal(aw_rec, aw_sum)
        wts_row = consts.tile([1, 9], F32)
        nc.vector.tensor_scalar_mul(wts_row, aw_exp, aw_rec)
        wtsT_ps = psum.tile([9, 1], F32, tag="sm")
        nc.tensor.transpose(wtsT_ps, wts_row, idf1)
        wtsT = consts.tile([9, 1], F32)
        nc.any.tensor_copy(wtsT, wtsT_ps)
        cw_ps = psum.tile([1, 512], F32, tag="l")
        nc.tensor.matmul(cw_ps[:, 0:5], wtsT, cmat_sb, start=True, stop=True)
        cw_row = consts.tile([1, 5], F32)
        nc.any.tensor_copy(cw_row, cw_ps[:, 0:5])
        bc_ps = psum.tile([128, 512], F32, tag="l")
        nc.tensor.matmul(bc_ps[:, 0:5], ones_f, cw_row, start=True, stop=True)
        coefs = consts.tile([128, 5], F32)
        nc.any.tensor_copy(coefs, bc_ps[:, 0:5])

        if with_bias:
            bias_b = consts.tile([1, UBF + NB], BF16)
            nc.gpsimd.dma_start(out=bias_b, in_=bias_d.ap()[None])

        # ---------------- Setup A2: connectivity (replicated) --------------
        h_ps = psum.tile([1, 512], F32, tag="l")
        for t in range(UK):
            nc.tensor.matmul(h_ps[:, 0:32], na_sb[:, t:t + 1], cw1_sb[:, t, :],
                             start=(t == 0), stop=(t == UK - 1))
        h_pre = consts.tile([1, 32], F32)
        nc.vector.tensor_add(h_pre, h_ps[:, 0:32], cb1_sb)
        h_sb = consts.tile([1, 32], F32)
        nc.scalar.activation(h_sb, h_pre, AF.Relu)
        hT_ps = psum.tile([32, 1], F32, tag="sm")
        nc.tensor.transpose(hT_ps, h_sb, idf1)
        hT_sb = consts.tile([32, 1], F32)
        nc.any.tensor_copy(hT_sb, hT_ps)
        cn_ps = psum.tile([1, 512], F32, tag="l")
        nc.tensor.matmul(cn_ps[:, 0:US], hT_sb, cw2_sb, start=True, stop=True)
        cn_pre = consts.tile([1, US], F32)
        nc.vector.tensor_add(cn_pre, cn_ps[:, 0:US], cb2_sb)
        cn_sig = consts.tile([1, US], F32)
        nc.scalar.activation(cn_sig, cn_pre, AF.Sigmoid)
        cm_row = consts.tile([1, US], F32)
        nc.vector.tensor_mul(cm_row, cn_sig, mask_sb)
        cm_ps = psum.tile([128, 512], F32, tag="l")
        nc.tensor.matmul(cm_ps[:, 0:US], ones_f, cm_row, start=True, stop=True)
        cm_bc = consts.tile([128, US], F32)
        nc.any.tensor_copy(cm_bc, cm_ps[:, 0:US])


        # ---------------- pipeline state ----------------
        ag_outs = [None] * NCH
        expTs = [None] * NCH
        rs_outs = [[None] * len(RS_GROUPS[ch]) for ch in range(NCH)]

        def phase1_tile(ch, t):
            csl = slice(t * 128, (t + 1) * 128)
            nsl = slice(ch * CW + t * 128, ch * CW + (t + 1) * 128)
            br = psum.tile([128, UBF], F32, tag="br", name="br")
            g_ps = psum.tile([128, NB], F32, tag="sm", name="g_ps")
            last = DK - 1
            for dk in range(DK):
                lhs = xt_c[ch][:, dk, csl]
                stop = (dk == last) and not with_bias
                # tiny gate MM first: the next dk's LDWEIGHTS then hides
                # under a 512-col stream instead of the tiny MM's drain
                nc.tensor.matmul(g_ps, lhs, wmod_sb[:, dk, UBF:UBF + NB],
                                 start=(dk == 0), stop=stop)
                nc.tensor.matmul(br[:, 0:512], lhs, wmod_sb[:, dk, 0:512],
                                 start=(dk == 0), stop=stop)
                nc.tensor.matmul(br[:, 512:1024], lhs,
                                 wmod_sb[:, dk, 512:1024],
                                 start=(dk == 0), stop=stop)
            if with_bias:
                nc.tensor.matmul(br[:, 0:512], ones_lhs, bias_b[:, 0:512],
                                 start=False, stop=True)
                nc.tensor.matmul(br[:, 512:1024], ones_lhs,
                                 bias_b[:, 512:1024], start=False, stop=True)
                nc.tensor.matmul(g_ps, ones_lhs, bias_b[:, UBF:UBF + NB],
                                 start=False, stop=True)
            # gate softmax
            g_negmax = blendp.tile([128, 1], F32, tag="g1")
            nc.vector.tensor_reduce(g_negmax, g_ps, AX.X, ALU.max, negate=True)
            g_exp = blendp.tile([128, NB], F32, tag="g2")
            g_sum = blendp.tile([128, 1], F32, tag="g3")
            nc.scalar.activation(g_exp, g_ps, AF.Exp, bias=g_negmax,
                                 accum_out=g_sum)
            g_rec = blendp.tile([128, 1], F32, tag="g4")
            nc.vector.reciprocal(g_rec, g_sum)
            # z*gsum = sum_b eg_b * branch_b (b-major contiguous slices);
            # the 1/gsum rides the relu's scale input below
            zt0 = blendp.tile([128, US], F32, tag="t0")
            nc.any.tensor_scalar_mul(zt0, br[:, 0:US], g_exp[:, 0:1])
            zt1 = blendp.tile([128, US], F32, tag="t1")
            nc.any.tensor_scalar_mul(zt1, br[:, US:2 * US], g_exp[:, 1:2])
            zt2 = blendp.tile([128, US], F32, tag="t2")
            nc.any.tensor_scalar_mul(zt2, br[:, 2 * US:3 * US],
                                     g_exp[:, 2:3])
            zt3 = blendp.tile([128, US], F32, tag="t3", bufs=1)
            nc.any.tensor_scalar_mul(zt3, br[:, 3 * US:4 * US],
                                     g_exp[:, 3:4])
            z01 = blendp.tile([128, US], F32, tag="t0")
            nc.any.tensor_add(z01, zt0, zt1)
            z23 = blendp.tile([128, US], F32, tag="t2")
            nc.any.tensor_add(z23, zt2, zt3)
            z_sb = blendp.tile([128, US], F32, tag="t1")
            nc.any.tensor_add(z_sb, z01, z23)
            # a = relu(z * conn * mask); bf16 for the Horner chain
            zc = blendp.tile([128, US], F32, tag="t0")
            nc.any.tensor_mul(zc, z_sb, cm_bc)
            a_sb = blendp.tile([128, US], BF16, tag="ta")
            nc.scalar.activation(a_sb, zc, AF.Relu, scale=g_rec)
            # blend via degree-4 Horner (per-partition scalar coefs)
            hp = blendp.tile([128, US], BF16, tag="h2")
            nc.any.tensor_scalar(hp, a_sb, coefs[:, 4:5], coefs[:, 3:4],
                                 ALU.mult, ALU.add)
            hq = blendp.tile([128, US], BF16, tag="h3", bufs=1)
            nc.any.tensor_mul(hq, hp, a_sb)
            hr = blendp.tile([128, US], BF16, tag="h2")
            nc.any.tensor_scalar_add(hr, hq, coefs[:, 2:3])
            hs = blendp.tile([128, US], BF16, tag="h3", bufs=1)
            nc.any.tensor_mul(hs, hr, a_sb)
            ht = blendp.tile([128, US], BF16, tag="h2")
            nc.any.tensor_scalar_add(ht, hs, coefs[:, 1:2])
            hu = blendp.tile([128, US], BF16, tag="h3", bufs=1)
            nc.any.tensor_mul(hu, ht, a_sb)
            blend_b16 = blendp.tile([128, US], BF16, tag="bb")
            nc.any.tensor_scalar_add(blend_b16, hu, coefs[:, 0:1])
            blends[t] = blend_b16

        def phase1_transpose(t):
            # PE transposes for tile t, emitted AFTER tile t+1's matmuls so
            # they never stall the PE on tile t's DVE blend chain
            csl = slice(t * 128, (t + 1) * 128)
            for uh in range(2):
                trb_ps = psum.tile([128, 128], BF16, tag="sm")
                nc.tensor.transpose(trb_ps,
                                    blends[t][:, uh * 128:(uh + 1) * 128], idb)
                nc.any.tensor_copy(blendT_c[:, uh, csl], trb_ps)

        def emit_allgather(ch, hf=None):
            # hf=None: whole chunk; hf=0/1: half-chunk (cols hf*256..)
            csl = slice(0, CW) if hf is None else slice(hf * 256,
                                                        (hf + 1) * 256)
            w = csl.stop - csl.start
            sfx = "" if hf is None else f"_{hf}"
            agi = dram_cc.tile([US, w], BF16, name=f"ag_in{ch}{sfx}",
                               tag=f"agi{ch}{sfx}")
            for uh in range(2):
                nc.sync.dma_start(out=agi[uh * 128:(uh + 1) * 128, :],
                                  in_=blendT_c[:, uh, csl])
            ago = dram_cc.tile([U, w], BF16, name=f"ag_out{ch}{sfx}",
                               tag=f"ago{ch}{sfx}", addr_space="Shared")
            nc.gpsimd.collective_compute(
                "AllGather", ALU.bypass,
                replica_groups=[list(range(KC))],
                ins=[agi.opt()], outs=[ago.opt()],
            )
            if hf is None:
                ag_outs[ch] = ago
            else:
                ag_outs[ch][hf] = ago

        def phase3(ch):
            bT = p34.tile([128, UK, CW], BF16, tag="bT", name="bT")
            for uk in range(UK):
                usl = slice(uk * 128, (uk + 1) * 128)
                # ch0's load is on the critical path: split across two rings
                eng = nc.scalar if (ch > 0 or uk % 2 == 0) else nc.sync
                eng.dma_start(out=bT[:, uk, :], in_=ag_outs[ch][usl, :])
            expT_t = p34.tile([128, MK, CW], BF16, tag="expT", name="expT_t")
            for mk in range(MK):
                l_ps = psum.tile([128, 512], F32, tag="l", name="l_ps")
                for uk in range(UK):
                    nc.tensor.matmul(l_ps,
                                     rw_sb[:, uk, mk * 128:(mk + 1) * 128],
                                     bT[:, uk, :],
                                     start=(uk == 0), stop=(uk == UK - 1))
                nc.scalar.activation(expT_t[:, mk, :], l_ps, AF.Exp,
                                     bias=rb_sb[:, mk:mk + 1])
            expTs[ch] = expT_t

        def phase4(ch):
            expT_t = expTs[ch]
            sj0 = 0
            for hf, spp in enumerate(RS_GROUPS[ch]):
                rs_inj = dram_cc.tile([spp * 128, MD + 1], BF16,
                                      name=f"rs_in{ch}_{hf}",
                                      tag=f"rsi{ch}{hf}")
                for sj2 in range(spp):
                    sj = sj0 + sj2
                    jsl = slice(sj * 128, (sj + 1) * 128)
                    r_ps = psum.tile([128, MD], F32, tag="br", name="r_ps")
                    rs_ps = psum.tile([128, 1], F32, tag="sm", name="rs_ps")
                    for mk in range(MK):
                        stat = expT_t[:, mk, jsl]
                        nc.tensor.matmul(rs_ps, stat,
                                         mem_sb[:, mk, MD:MD + 1],
                                         start=(mk == 0), stop=(mk == MK - 1))
                        nc.tensor.matmul(r_ps[:, 0:512], stat,
                                         mem_sb[:, mk, 0:512],
                                         start=(mk == 0), stop=(mk == MK - 1))
                        nc.tensor.matmul(r_ps[:, 512:1024], stat,
                                         mem_sb[:, mk, 512:1024],
                                         start=(mk == 0), stop=(mk == MK - 1))
                    r_sb = p34.tile([128, MD + 1], BF16, tag="rsb",
                                    name="r_sb")
                    nc.any.tensor_copy(r_sb[:, 0:MD], r_ps)
                    nc.any.tensor_copy(r_sb[:, MD:MD + 1], rs_ps)
                    nc.sync.dma_start(out=rs_inj[sj2 * 128:(sj2 + 1) * 128, :],
                                      in_=r_sb)
                rs_out = dram_cc.tile([spp * 128 // KC, MD + 1], BF16,
                                      name=f"rs_out{ch}_{hf}",
                                      tag=f"rso{ch}{hf}")
                nc.gpsimd.collective_compute(
                    "ReduceScatter", ALU.add,
                    replica_groups=[list(range(KC))],
                    ins=[rs_inj.opt()], outs=[rs_out.opt()],
                )
                rs_outs[ch][hf] = rs_out
                sj0 += spp

        def epilogue(ch, hf):
            rows = RS_GROUPS[ch][hf] * 128 // KC
            e_f = p34.tile([rows, MD + 1], BF16, tag="ef", name="e_f")
            nc.gpsimd.dma_start(out=e_f, in_=rs_outs[ch][hf][:, :])
            s32 = p34.tile([rows, 1], F32, tag="s32", name="s32")
            nc.any.tensor_copy(s32, e_f[:, MD:MD + 1])
            s_rec = p34.tile([rows, 1], F32, tag="sr", name="s_rec")
            nc.vector.reciprocal(s_rec, s32)
            y_t = p34.tile([rows, MD], F32, tag="yt", name="y_t")
            nc.any.tensor_scalar_mul(y_t, e_f[:, 0:MD], s_rec)
            r0 = ch * 64 + sum(RS_GROUPS[ch][:hf]) * 128 // KC
            nc.gpsimd.dma_start(out=y_d[r0:r0 + rows, :], in_=y_t)

        # ---------------- main pipeline ----------------
        for ch in range(NCH):
            if ch + 1 < NCH:
                xt_c[ch + 1] = st.tile([128, DK, CW], BF16, tag="xt",
                                       name=f"xt_c{ch + 1}")
                for dk in range(DK):
                    nc.scalar.dma_start(
                        out=xt_c[ch + 1][:, dk, :],
                        in_=xt_d[dk * 128:(dk + 1) * 128,
                                 (ch + 1) * CW:(ch + 2) * CW])
            if ch == 1:
                # bulk phase-3/4 loads mid-phase-1: after the critical xt
                # loads, clear of the AG0 window
                for uk in range(UK):
                    nc.scalar.dma_start(out=rw_sb[:, uk, :],
                                        in_=rw_d[uk * 128:(uk + 1) * 128, :])
                for mk in range(MK):
                    nc.scalar.dma_start(out=mem_sb[:, mk, 0:MD],
                                        in_=mem_d[mk * 128:(mk + 1) * 128, :])
                    nc.vector.memset(mem_sb[:, mk, MD:MD + 1], 1.0)
            blendT_c = blendp.tile([128, 2, CW], BF16, tag="bl",
                                   name=f"blendT{ch}")
            blends = [None] * 4
            for t in range(4):
                phase1_tile(ch, t)
                if t >= 1:
                    phase1_transpose(t - 1)
            phase1_transpose(3)
            emit_allgather(ch)

        # all AG triggers precede all RS triggers on the collective queue,
        # so no early collective is blocked behind a late one's staging
        for ch in range(NCH):
            phase3(ch)
            phase4(ch)
        # epilogues after all compute: their RS results (except ch3's) are
        # ready, and no collective trigger queues behind their waits
        for ch in range(NCH):
            for hf in range(len(RS_GROUPS[ch])):
                epilogue(ch, hf)

    nc.compile()
    return nc


def _make_in_maps(inputs):
    bf = ml_dtypes.bfloat16
    x = np.asarray(inputs["x"], np.float32)
    w = np.asarray(inputs["w"], np.float32)
    delay = np.asarray(inputs["delay"], np.float32)
    b = np.asarray(inputs["b"], np.float32)
    gate_W = np.asarray(inputs["gate_W"], np.float32)
    gate_b = np.asarray(inputs["gate_b"], np.float32)
    na = np.ascontiguousarray(np.asarray(inputs["neuron_avg"], np.float32))
    cw1 = np.ascontiguousarray(np.asarray(inputs["conn_W1"], np.float32))
    cb1 = np.ascontiguousarray(np.asarray(inputs["conn_b1"], np.float32))
    cw2 = np.asarray(inputs["conn_W2"], np.float32)
    cb2 = np.asarray(inputs["conn_b2"], np.float32)
    mask = np.asarray(inputs["mask"], np.float32)
    actw = np.ascontiguousarray(np.asarray(inputs["act_w"], np.float32))
    read_W = np.asarray(inputs["read_W"], np.float32)
    read_b = np.asarray(inputs["read_b"], np.float32)
    mem = np.asarray(inputs["memory"], np.float32)

    xt = np.ascontiguousarray(x.T).astype(bf)
    in_maps = []
    for k in range(KC):
        us, ue = k * US, (k + 1) * US
        ms, me = k * MS, (k + 1) * MS
        bias_row = np.concatenate([b[us:ue].T.reshape(-1),
                                   gate_b]).astype(np.float32)
        in_maps.append({
            "xt": xt,
            "wd": np.ascontiguousarray(np.concatenate(
                [w[:, us:ue, :].transpose(0, 2, 1).reshape(D, UBF), gate_W],
                axis=1)).astype(bf),
            "dd": np.ascontiguousarray(
                delay[:, us:ue, :].transpose(0, 2, 1).reshape(D, UBF)).astype(bf),
            "bias": np.ascontiguousarray(bias_row),
            "na": na,
            "cw1": cw1,
            "cb1": cb1,
            "cw2": np.ascontiguousarray(cw2[:, us:ue]),
            "cb2": np.ascontiguousarray(cb2[us:ue]),
            "maskv": np.ascontiguousarray(mask[us:ue]),
            "actw": actw,
            "rw": np.ascontiguousarray(read_W[:, ms:me]).astype(bf),
            "rb": np.ascontiguousarray(read_b[ms:me]),
            "mem": np.ascontiguousarray(mem[ms:me, :]).astype(bf),
            "cmat": _CMAT,
        })
    return in_maps


def kernel(**inputs) -> np.ndarray:
    with_bias = bool(np.any(np.asarray(inputs["b"]))
                     or np.any(np.asarray(inputs["gate_b"])))
    key = ("nc", with_bias)
    if key not in _cache:
        _cache[key] = _build(with_bias)
        _cache["nc"] = _cache[key]
    nc = _cache[key]
    in_maps = _make_in_maps(inputs)
    res = run_bass_kernel_spmd(nc, in_maps, core_ids=list(range(KC)))
    out = np.empty((N, MD), np.float32)
    for k in range(KC):
        yk = res.results[k]["y"]
        for ch in range(4):
            base = 0
            for spp in RS_GROUPS[ch]:
                rows = spp * 128 // KC
                g0 = ch * 512 + base * 128 + k * rows
                l0 = ch * 64 + base * 128 // KC
                out[g0:g0 + rows] = yk[l0:l0 + rows]
                base += spp
    return out


# revision 29
# speedup vs baseline: 1.0262x; 1.0262x over previous
"""Trainium2 Bass kernel for nn_PlasticityModelMoE (8-core SPMD), v2.

Strategy:
  Units tensor-parallel phase 1 (256 units/core): host supplies xT (bf16)
  and b-major w/delay (bf16); wmod = w*sigmoid(delay) on device. Per
  128-row batch tile, dk-outer matmul loop shares each xT stationary
  across the two 512-col branch groups + the 4-col gate group. Gate
  softmax (ACT Exp with accum_out, 1/sum folded into the relu's scale),
  z weighted sum, conn*mask, ReLU, degree-4 Horner blend in bf16, PE
  transposes (emitted one tile late so they never stall on the DVE
  chain).
  Schedule: tiny warmup collective at t=0 absorbs the ~40-80us CC-stream
  bring-up + inter-core launch skew; all four phase-1 chunks run first
  (their AllGather triggers all precede any ReduceScatter trigger on the
  in-order collective queue); then per chunk: phase 3 (memory-rows TP,
  1024 rows/core): logitsT = read_W x blendT, expT = exp(logitsT +
  read_b); phase 4: [read | s] = expT @ [mem | 1] with mk-outer shared
  stationaries (tiny s-column matmul first so the next weight load hides
  under a 512-col stream); bf16 ReduceScatter over batch rows (whole-
  chunk for ch0-2, halves for ch3 so the tail only exposes one half);
  all epilogues (divide by s, store 32/64-row slices) run at the end.
"""
import numpy as np
import ml_dtypes
from contextlib import ExitStack

import concourse.bass as bass
import concourse.mybir as mybir
import concourse.tile as tile
from concourse import bacc
from concourse.bass_utils import run_bass_kernel_spmd
from concourse.masks import make_identity

F32 = mybir.dt.float32
BF16 = mybir.dt.bfloat16
AF = mybir.ActivationFunctionType
ALU = mybir.AluOpType
AX = mybir.AxisListType

KC = 8
N, D, U, NB, M, MD = 2048, 1024, 2048, 4, 8192, 1024
US = U // KC          # 256 units per core
MS = M // KC          # 1024 memory rows per core
NS = N // KC          # 256 output rows per core
DK = D // 128         # 8 k-tiles over D
UK = U // 128         # 16 k-tiles over U
MK = MS // 128        # 8 k-tiles over memory shard
UBF = US * NB         # 1024 branch columns per core
NCH = 4               # batch chunks
CW = N // NCH         # 512 columns per chunk
RS_GROUPS = [[4], [4], [4], [3, 1]]  # sj per ReduceScatter, per chunk

_CMAT = np.array([
    [5.0000238e-01, 2.4987496e-01, 1.0582031e-03, -2.4046743e-02, 4.1678566e-03],
    [0.0, 1.0, 0.0, 0.0, 0.0],
    [-7.2632770e-06, 9.9976927e-01, 9.2018498e-03, -3.9401752e-01, 1.4669961e-01],
    [0.0, 1.0, 0.0, 0.0, 0.0],
    [8.6798245e-06, 4.9957812e-01, 2.5321743e-01, -8.1970906e-03, -1.3558048e-02],
    [3.9388153e-05, 4.9807969e-01, 4.1364601e-01, -3.7666172e-02, -3.2796454e-02],
    [0.0, 1.0507009873554805, 0.0, 0.0, 0.0],
    [3.1482985e-05, 5.9846270e-01, 3.3178753e-01, -4.6201140e-02, -1.9015398e-02],
    [0.0, 0.0, 0.0, 0.0, 0.0],
], dtype=np.float32)

_cache = {}


def _build(with_bias):
    nc = bacc.Bacc(num_devices=KC)

    xt_d = nc.dram_tensor("xt", [D, N], BF16, kind="ExternalInput")
    wd_d = nc.dram_tensor("wd", [D, UBF + NB], BF16, kind="ExternalInput")
    dd_d = nc.dram_tensor("dd", [D, UBF], BF16, kind="ExternalInput")
    bias_d = nc.dram_tensor("bias", [UBF + NB], F32, kind="ExternalInput")
    na_d = nc.dram_tensor("na", [U], F32, kind="ExternalInput")
    cw1_d = nc.dram_tensor("cw1", [U, 32], F32, kind="ExternalInput")
    cb1_d = nc.dram_tensor("cb1", [32], F32, kind="ExternalInput")
    cw2_d = nc.dram_tensor("cw2", [32, US], F32, kind="ExternalInput")
    cb2_d = nc.dram_tensor("cb2", [US], F32, kind="ExternalInput")
    mask_d = nc.dram_tensor("maskv", [US], F32, kind="ExternalInput")
    actw_d = nc.dram_tensor("actw", [9], F32, kind="ExternalInput")
    rw_d = nc.dram_tensor("rw", [U, MS], BF16, kind="ExternalInput")
    rb_d = nc.dram_tensor("rb", [MS], F32, kind="ExternalInput")
    mem_d = nc.dram_tensor("mem", [MS, MD], BF16, kind="ExternalInput")
    cmat_d = nc.dram_tensor("cmat", [9, 5], F32, kind="ExternalInput")
    y_d = nc.dram_tensor("y", [NS, MD], F32, kind="ExternalOutput")

    with tile.TileContext(nc) as tc, ExitStack() as ctx:
        consts = ctx.enter_context(tc.tile_pool(name="consts", bufs=1))
        big = ctx.enter_context(tc.tile_pool(name="big", bufs=1))
        st = ctx.enter_context(tc.tile_pool(name="st", bufs=2))
        blendp = ctx.enter_context(tc.tile_pool(name="blendp", bufs=2))
        p34 = ctx.enter_context(tc.tile_pool(name="p34", bufs=2))
        dram_cc = ctx.enter_context(tc.tile_pool(name="dram_cc", bufs=1,
                                                 space="DRAM"))
        # PSUM budget (8 banks, bank-granular per tag-buf):
        #   br [128,1024] f32 = 2 banks x2 = 4  (ph1 branch; ph4 read)
        #   l  [128,512]  f32 = 1 bank  x2 = 2  (setup misc; ph3 logits)
        #   sm [128,<=128]    = 1 bank  x2 = 2  (transposes, gate, s col)
        psum = ctx.enter_context(tc.tile_pool(name="psum", bufs=2, space="PSUM"))

        # -------- tiny setup DMAs first so setup PE work starts at t~0 -----
        aw = consts.tile([1, 9], F32)
        nc.sync.dma_start(out=aw, in_=actw_d.ap()[None])
        cmat_sb = consts.tile([9, 5], F32)
        nc.sync.dma_start(out=cmat_sb, in_=cmat_d[:, :])
        na_sb = consts.tile([128, UK], F32)
        nc.sync.dma_start(out=na_sb, in_=na_d.ap().rearrange("(t p) -> p t", p=128))
        cw1_sb = consts.tile([128, UK, 32], F32)
        nc.sync.dma_start(out=cw1_sb,
                          in_=cw1_d.ap().rearrange("(t p) c -> p t c", p=128))
        cb1_sb = consts.tile([1, 32], F32)
        nc.sync.dma_start(out=cb1_sb, in_=cb1_d.ap()[None])
        cw2_sb = consts.tile([32, US], F32)
        nc.sync.dma_start(out=cw2_sb, in_=cw2_d[:, :])
        cb2_sb = consts.tile([1, US], F32)
        nc.sync.dma_start(out=cb2_sb, in_=cb2_d.ap()[None])
        mask_sb = consts.tile([1, US], F32)
        nc.sync.dma_start(out=mask_sb, in_=mask_d.ap()[None])
        rb_sb = consts.tile([128, MK], F32)
        nc.sync.dma_start(out=rb_sb, in_=rb_d.ap().rearrange("(t p) -> p t", p=128))

        # tiny warmup collective: absorbs the ~40us first-collective cost
        # (CC stream bring-up + inter-core launch skew) off the AG0 path
        warm_pool = ctx.enter_context(tc.tile_pool(name="warm_pool", bufs=1,
                                                   space="DRAM"))
        warm_sb = consts.tile([1, 16], BF16)
        nc.vector.memset(warm_sb, 0.0)
        warm_in = warm_pool.tile([1, 16], BF16, name="warm_in", tag="wi")
        nc.gpsimd.dma_start(out=warm_in, in_=warm_sb)
        warm_out = warm_pool.tile([KC, 16], BF16, name="warm_out", tag="wo",
                                  addr_space="Shared")
        nc.gpsimd.collective_compute(
            "AllGather", ALU.bypass, replica_groups=[list(range(KC))],
            ins=[warm_in.opt()], outs=[warm_out.opt()],
        )

        # ---------------- startup DMAs: wd/dd/xt(ch0) on sync queue --------
        wd_s = []
        dd_s = []
        xt_c = [None] * NCH
        xt_c[0] = st.tile([128, DK, CW], BF16, tag="xt", name="xt_c0")
        for dk in range(DK):
            w_b = st.tile([128, UBF + NB], BF16, tag="lw", name=f"w_b{dk}")
            nc.sync.dma_start(out=w_b, in_=wd_d[dk * 128:(dk + 1) * 128, :])
            d_b = st.tile([128, UBF], BF16, tag="ld", name=f"d_b{dk}")
            nc.sync.dma_start(out=d_b, in_=dd_d[dk * 128:(dk + 1) * 128, :])
            nc.sync.dma_start(out=xt_c[0][:, dk, :],
                              in_=xt_d[dk * 128:(dk + 1) * 128, 0:CW])
            wd_s.append(w_b)
            dd_s.append(d_b)

        rw_sb = big.tile([128, UK, MS], BF16)
        mem_sb = big.tile([128, MK, MD + 1], BF16)

        # ---------------- wmod = w * sigmoid(delay), bf16 b-major ----------
        wmod_sb = big.tile([128, DK, UBF + NB], BF16)
        for dk in range(DK):
            sig_b = st.tile([128, UBF], BF16, tag="sg", name=f"sig{dk}")
            nc.scalar.activation(sig_b, dd_s[dk], AF.Sigmoid)
            nc.vector.tensor_mul(wmod_sb[:, dk, 0:UBF], wd_s[dk][:, 0:UBF],
                                 sig_b)
            nc.any.tensor_copy(wmod_sb[:, dk, UBF:UBF + NB],
                               wd_s[dk][:, UBF:UBF + NB])

        # ---------------- Setup A: tiny constants ----------------
        idf = consts.tile([128, 128], F32)
        make_identity(nc, idf)
        idb = consts.tile([128, 128], BF16)
        nc.any.tensor_copy(idb, idf)
        ones_lhs = consts.tile([1, 128], BF16)
        nc.vector.memset(ones_lhs, 1.0)
        ones_f = consts.tile([1, 128], F32)
        nc.vector.memset(ones_f, 1.0)
        idf1 = consts.tile([1, 1], F32)
        nc.vector.memset(idf1, 1.0)

        # softmax(act_w); polynomial coefs = wts @ cmat, broadcast to [128, 5]
        aw_negmax = consts.tile([1, 1], F32)
        nc.vector.tensor_reduce(aw_negmax, aw, AX.X, ALU.max, negate=True)
        aw_exp = consts.tile([1, 9], F32)
        aw_sum = consts.tile([1, 1], F32)
        nc.scalar.activation(aw_exp, aw, AF.Exp, bias=aw_negmax,
                             accum_out=aw_sum)
        aw_rec = consts.tile([1, 1], F32)
        nc.vector.reciprocal(aw_rec, aw_sum)
        wts_row = consts.tile([1, 9], F32)
        nc.vector.tensor_scalar_mul(wts_row, aw_exp, aw_rec)
        wtsT_ps = psum.tile([9, 1], F32, tag="sm")
        nc.tensor.transpose(wtsT_ps, wts_row, idf1)
        wtsT = consts.tile([9, 1], F32)
        nc.any.tensor_copy(wtsT, wtsT_ps)
        cw_ps = psum.tile([1, 512], F32, tag="l")
        nc.tensor.matmul(cw_ps[:, 0:5], wtsT, cmat_sb, start=True, stop=True)
        cw_row = consts.tile([1, 5], F32)
        nc.any.tensor_copy(cw_row, cw_ps[:, 0:5])
        bc_ps = psum.tile([128, 512], F32, tag="l")
        nc.tensor.matmul(bc_ps[:, 0:5], ones_f, cw_row, start=True, stop=True)
        coefs = consts.tile([128, 5], F32)
        nc.any.tensor_copy(coefs, bc_ps[:, 0:5])

        if with_bias:
            bias_b = consts.tile([1, UBF + NB], BF16)
            nc.gpsimd.dma_start(out=bias_b, in_=bias_d.ap()[None])

        # ---------------- Setup A2: connectivity (replicated) --------------
        h_ps = psum.tile([1, 512], F32, tag="l")
        for t in range(UK):
            nc.tensor.matmul(h_ps[:, 0:32], na_sb[:, t:t + 1], cw1_sb[:, t, :],
                             start=(t == 0), stop=(t == UK - 1))
        h_pre = consts.tile([1, 32], F32)
        nc.vector.tensor_add(h_pre, h_ps[:, 0:32], cb1_sb)
        h_sb = consts.tile([1, 32], F32)
        nc.scalar.activation(h_sb, h_pre, AF.Relu)
        hT_ps = psum.tile([32, 1], F32, tag="sm")
        nc.tensor.transpose(hT_ps, h_sb, idf1)
        hT_sb = consts.tile([32, 1], F32)
        nc.any.tensor_copy(hT_sb, hT_ps)
        cn_ps = psum.tile([1, 512], F32, tag="l")
        nc.tensor.matmul(cn_ps[:, 0:US], hT_sb, cw2_sb, start=True, stop=True)
        cn_pre = consts.tile([1, US], F32)
        nc.vector.tensor_add(cn_pre, cn_ps[:, 0:US], cb2_sb)
        cn_sig = consts.tile([1, US], F32)
        nc.scalar.activation(cn_sig, cn_pre, AF.Sigmoid)
        cm_row = consts.tile([1, US], F32)
        nc.vector.tensor_mul(cm_row, cn_sig, mask_sb)
        cm_ps = psum.tile([128, 512], F32, tag="l")
        nc.tensor.matmul(cm_ps[:, 0:US], ones_f, cm_row, start=True, stop=True)
        cm_bc = consts.tile([128, US], F32)
        nc.any.tensor_copy(cm_bc, cm_ps[:, 0:US])


        # ---------------- pipeline state ----------------
        ag_outs = [None] * NCH
        expTs = [None] * NCH
        rs_outs = [[None] * len(RS_GROUPS[ch]) for ch in range(NCH)]

        def phase1_tile(ch, t):
            csl = slice(t * 128, (t + 1) * 128)
            nsl = slice(ch * CW + t * 128, ch * CW + (t + 1) * 128)
            br = psum.tile([128, UBF], F32, tag="br", name="br")
            g_ps = psum.tile([128, NB], F32, tag="sm", name="g_ps")
            last = DK - 1
            for dk in range(DK):
                lhs = xt_c[ch][:, dk, csl]
                stop = (dk == last) and not with_bias
                # tiny gate MM first: the next dk's LDWEIGHTS then hides
                # under a 512-col stream instead of the tiny MM's drain
                nc.tensor.matmul(g_ps, lhs, wmod_sb[:, dk, UBF:UBF + NB],
                                 start=(dk == 0), stop=stop)
                nc.tensor.matmul(br[:, 0:512], lhs, wmod_sb[:, dk, 0:512],
                                 start=(dk == 0), stop=stop)
                nc.tensor.matmul(br[:, 512:1024], lhs,
                                 wmod_sb[:, dk, 512:1024],
                                 start=(dk == 0), stop=stop)
            if with_bias:
                nc.tensor.matmul(br[:, 0:512], ones_lhs, bias_b[:, 0:512],
                                 start=False, stop=True)
                nc.tensor.matmul(br[:, 512:1024], ones_lhs,
                                 bias_b[:, 512:1024], start=False, stop=True)
                nc.tensor.matmul(g_ps, ones_lhs, bias_b[:, UBF:UBF + NB],
                                 start=False, stop=True)
            # gate softmax
            g_negmax = blendp.tile([128, 1], F32, tag="g1")
            nc.vector.tensor_reduce(g_negmax, g_ps, AX.X, ALU.max, negate=True)
            g_exp = blendp.tile([128, NB], F32, tag="g2")
            g_sum = blendp.tile([128, 1], F32, tag="g3")
            nc.scalar.activation(g_exp, g_ps, AF.Exp, bias=g_negmax,
                                 accum_out=g_sum)
            g_rec = blendp.tile([128, 1], F32, tag="g4")
            nc.vector.reciprocal(g_rec, g_sum)
            # z*gsum = sum_b eg_b * branch_b (b-major contiguous slices);
            # the 1/gsum rides the relu's scale input below
            zt0 = blendp.tile([128, US], F32, tag="t0")
            nc.any.tensor_scalar_mul(zt0, br[:, 0:US], g_exp[:, 0:1])
            zt1 = blendp.tile([128, US], F32, tag="t1")
            nc.any.tensor_scalar_mul(zt1, br[:, US:2 * US], g_exp[:, 1:2])
            zt2 = blendp.tile([128, US], F32, tag="t2")
            nc.any.tensor_scalar_mul(zt2, br[:, 2 * US:3 * US],
                                     g_exp[:, 2:3])
            zt3 = blendp.tile([128, US], F32, tag="t3", bufs=1)
            nc.any.tensor_scalar_mul(zt3, br[:, 3 * US:4 * US],
                                     g_exp[:, 3:4])
            z01 = blendp.tile([128, US], F32, tag="t0")
            nc.any.tensor_add(z01, zt0, zt1)
            z23 = blendp.tile([128, US], F32, tag="t2")
            nc.any.tensor_add(z23, zt2, zt3)
            z_sb = blendp.tile([128, US], F32, tag="t1")
            nc.any.tensor_add(z_sb, z01, z23)
            # a = relu(z * conn * mask); bf16 for the Horner chain
            zc = blendp.tile([128, US], F32, tag="t0")
            nc.any.tensor_mul(zc, z_sb, cm_bc)
            a_sb = blendp.tile([128, US], BF16, tag="ta")
            nc.scalar.activation(a_sb, zc, AF.Relu, scale=g_rec)
            # blend via degree-4 Horner (per-partition scalar coefs)
            hp = blendp.tile([128, US], BF16, tag="h2")
            nc.any.tensor_scalar(hp, a_sb, coefs[:, 4:5], coefs[:, 3:4],
                                 ALU.mult, ALU.add)
            hq = blendp.tile([128, US], BF16, tag="h3", bufs=1)
            nc.any.tensor_mul(hq, hp, a_sb)
            hr = blendp.tile([128, US], BF16, tag="h2")
            nc.any.tensor_scalar_add(hr, hq, coefs[:, 2:3])
            hs = blendp.tile([128, US], BF16, tag="h3", bufs=1)
            nc.any.tensor_mul(hs, hr, a_sb)
            ht = blendp.tile([128, US], BF16, tag="h2")
            nc.any.tensor_scalar_add(ht, hs, coefs[:, 1:2])
            hu = blendp.tile([128, US], BF16, tag="h3", bufs=1)
            nc.any.tensor_mul(hu, ht, a_sb)
            blend_b16 = blendp.tile([128, US], BF16, tag="bb")
            nc.any.tensor_scalar_add(blend_b16, hu, coefs[:, 0:1])
            blends[t] = blend_b16

        def phase1_transpose(t):
            # PE transposes for tile t, emitted AFTER tile t+1's matmuls so
            # they never stall the PE on tile t's DVE blend chain
            csl = slice(t * 128, (t + 1) * 128)
            for uh in range(2):
                trb_ps = psum.tile([128, 128], BF16, tag="sm")
                nc.tensor.transpose(trb_ps,
                                    blends[t][:, uh * 128:(uh + 1) * 128], idb)
                nc.any.tensor_copy(blendT_c[:, uh, csl], trb_ps)

        def emit_allgather(ch, hf=None):
            # hf=None: whole chunk; hf=0/1: half-chunk (cols hf*256..)
            csl = slice(0, CW) if hf is None else slice(hf * 256,
                                                        (hf + 1) * 256)
            w = csl.stop - csl.start
            sfx = "" if hf is None else f"_{hf}"
            agi = dram_cc.tile([US, w], BF16, name=f"ag_in{ch}{sfx}",
                               tag=f"agi{ch}{sfx}")
            for uh in range(2):
                nc.sync.dma_start(out=agi[uh * 128:(uh + 1) * 128, :],
                                  in_=blendT_c[:, uh, csl])
            ago = dram_cc.tile([U, w], BF16, name=f"ag_out{ch}{sfx}",
                               tag=f"ago{ch}{sfx}", addr_space="Shared")
            nc.gpsimd.collective_compute(
                "AllGather", ALU.bypass,
                replica_groups=[list(range(KC))],
                ins=[agi.opt()], outs=[ago.opt()],
            )
            if hf is None:
                ag_outs[ch] = ago
            else:
                ag_outs[ch][hf] = ago

        def phase3(ch):
            bT = p34.tile([128, UK, CW], BF16, tag="bT", name="bT")
            for uk in range(UK):
                usl = slice(uk * 128, (uk + 1) * 128)
                # ch0's load is on the critical path: split across two rings
                eng = nc.scalar if (ch > 0 or uk % 2 == 0) else nc.sync
                eng.dma_start(out=bT[:, uk, :], in_=ag_outs[ch][usl, :])
            expT_t = p34.tile([128, MK, CW], BF16, tag="expT", name="expT_t")
            for mk in range(MK):
                l_ps = psum.tile([128, 512], F32, tag="l", name="l_ps")
                for uk in range(UK):
                    nc.tensor.matmul(l_ps,
                                     rw_sb[:, uk, mk * 128:(mk + 1) * 128],
                                     bT[:, uk, :],
                                     start=(uk == 0), stop=(uk == UK - 1))
                nc.scalar.activation(expT_t[:, mk, :], l_ps, AF.Exp,
                                     bias=rb_sb[:, mk:mk + 1])
            expTs[ch] = expT_t

        def phase4(ch):
            expT_t = expTs[ch]
            sj0 = 0
            for hf, spp in enumerate(RS_GROUPS[ch]):
                rs_inj = dram_cc.tile([spp * 128, MD + 1], BF16,
                                      name=f"rs_in{ch}_{hf}",
                                      tag=f"rsi{ch}{hf}")
                for sj2 in range(spp):
                    sj = sj0 + sj2
                    jsl = slice(sj * 128, (sj + 1) * 128)
                    r_ps = psum.tile([128, MD], F32, tag="br", name="r_ps")
                    rs_ps = psum.tile([128, 1], F32, tag="sm", name="rs_ps")
                    for mk in range(MK):
                        stat = expT_t[:, mk, jsl]
                        nc.tensor.matmul(rs_ps, stat,
                                         mem_sb[:, mk, MD:MD + 1],
                                         start=(mk == 0), stop=(mk == MK - 1))
                        nc.tensor.matmul(r_ps[:, 0:512], stat,
                                         mem_sb[:, mk, 0:512],
                                         start=(mk == 0), stop=(mk == MK - 1))
                        nc.tensor.matmul(r_ps[:, 512:1024], stat,
                                         mem_sb[:, mk, 512:1024],
                                         start=(mk == 0), stop=(mk == MK - 1))
                    r_sb = p34.tile([128, MD + 1], BF16, tag="rsb",
                                    name="r_sb")
                    nc.any.tensor_copy(r_sb[:, 0:MD], r_ps)
                    nc.any.tensor_copy(r_sb[:, MD:MD + 1], rs_ps)
                    nc.sync.dma_start(out=rs_inj[sj2 * 128:(sj2 + 1) * 128, :],
                                      in_=r_sb)
                rs_out = dram_cc.tile([spp * 128 // KC, MD + 1], BF16,
                                      name=f"rs_out{ch}_{hf}",
                                      tag=f"rso{ch}{hf}")
                nc.gpsimd.collective_compute(
                    "ReduceScatter", ALU.add,
                    replica_groups=[list(range(KC))],
                    ins=[rs_inj.opt()], outs=[rs_out.opt()],
                )
                rs_outs[ch][hf] = rs_out
                sj0 += spp

        def epilogue(ch, hf):
            rows = RS_GROUPS[ch][hf] * 128 // KC
            e_f = p34.tile([rows, MD + 1], BF16, tag="ef", name="e_f")
            nc.gpsimd.dma_start(out=e_f, in_=rs_outs[ch][hf][:, :])
            s32 = p34.tile([rows, 1], F32, tag="s32", name="s32")
            nc.any.tensor_copy(s32, e_f[:, MD:MD + 1])
            s_rec = p34.tile([rows, 1], F32, tag="sr", name="s_rec")
            nc.vector.reciprocal(s_rec, s32)
            y_t = p34.tile([rows, MD], F32, tag="yt", name="y_t")
            nc.any.tensor_scalar_mul(y_t, e_f[:, 0:MD], s_rec)
            r0 = ch * 64 + sum(RS_GROUPS[ch][:hf]) * 128 // KC
            nc.gpsimd.dma_start(out=y_d[r0:r0 + rows, :], in_=y_t)

        # ---------------- main pipeline ----------------
        for ch in range(NCH):
            if ch + 1 < NCH:
                xt_c[ch + 1] = st.tile([128, DK, CW], BF16, tag="xt",
                                       name=f"xt_c{ch + 1}")
                for dk in range(DK):
                    nc.scalar.dma_start(
                        out=xt_c[ch + 1][:, dk, :],
                        in_=xt_d[dk * 128:(dk + 1) * 128,
                                 (ch + 1) * CW:(ch + 2) * CW])
            if ch == 1:
                # bulk phase-3/4 loads mid-phase-1: after the critical xt
                # loads, clear of the AG0 window
                for uk in range(UK):
                    nc.scalar.dma_start(out=rw_sb[:, uk, :],
                                        in_=rw_d[uk * 128:(uk + 1) * 128, :])
                for mk in range(MK):
                    nc.scalar.dma_start(out=mem_sb[:, mk, 0:MD],
                                        in_=mem_d[mk * 128:(mk + 1) * 128, :])
                    nc.vector.memset(mem_sb[:, mk, MD:MD + 1], 1.0)
            blendT_c = blendp.tile([128, 2, CW], BF16, tag="bl",
                                   name=f"blendT{ch}")
            blends = [None] * 4
            for t in range(4):
                phase1_tile(ch, t)
                if t >= 1:
                    phase1_transpose(t - 1)
            phase1_transpose(3)
            emit_allgather(ch)

        # all AG triggers precede all RS triggers on the collective queue,
        # so no early collective is blocked behind a late one's staging
        for ch in range(NCH):
            phase3(ch)
            phase4(ch)
        # epilogues after all compute: their RS results (except ch3's) are
        # ready, and no collective trigger queues behind their waits
        for ch in range(NCH):
            for hf in range(len(RS_GROUPS[ch])):
                epilogue(ch, hf)

    nc.compile()
    return nc


def _make_in_maps(inputs):
    bf = ml_dtypes.bfloat16
    x = np.asarray(inputs["x"], np.float32)
    w = np.asarray(inputs["w"], np.float32)
    delay = np.asarray(inputs["delay"], np.float32)
    b = np.asarray(inputs["b"], np.float32)
    gate_W = np.asarray(inputs["gate_W"], np.float32)
    gate_b = np.asarray(inputs["gate_b"], np.float32)
    na = np.ascontiguousarray(np.asarray(inputs["neuron_avg"], np.float32))
    cw1 = np.ascontiguousarray(np.asarray(inputs["conn_W1"], np.float32))
    cb1 = np.ascontiguousarray(np.asarray(inputs["conn_b1"], np.float32))
    cw2 = np.asarray(inputs["conn_W2"], np.float32)
    cb2 = np.asarray(inputs["conn_b2"], np.float32)
    mask = np.asarray(inputs["mask"], np.float32)
    actw = np.ascontiguousarray(np.asarray(inputs["act_w"], np.float32))
    read_W = np.asarray(inputs["read_W"], np.float32)
    read_b = np.asarray(inputs["read_b"], np.float32)
    mem = np.asarray(inputs["memory"], np.float32)

    xt = np.ascontiguousarray(x.T).astype(bf)
    in_maps = []
    for k in range(KC):
        us, ue = k * US, (k + 1) * US
        ms, me = k * MS, (k + 1) * MS
        bias_row = np.concatenate([b[us:ue].T.reshape(-1),
                                   gate_b]).astype(np.float32)
        in_maps.append({
            "xt": xt,
            "wd": np.ascontiguousarray(np.concatenate(
                [w[:, us:ue, :].transpose(0, 2, 1).reshape(D, UBF), gate_W],
                axis=1)).astype(bf),
            "dd": np.ascontiguousarray(
                delay[:, us:ue, :].transpose(0, 2, 1).reshape(D, UBF)).astype(bf),
            "bias": np.ascontiguousarray(bias_row),
            "na": na,
            "cw1": cw1,
            "cb1": cb1,
            "cw2": np.ascontiguousarray(cw2[:, us:ue]),
            "cb2": np.ascontiguousarray(cb2[us:ue]),
            "maskv": np.ascontiguousarray(mask[us:ue]),
            "actw": actw,
            "rw": np.ascontiguousarray(read_W[:, ms:me]).astype(bf),
            "rb": np.ascontiguousarray(read_b[ms:me]),
            "mem": np.ascontiguousarray(mem[ms:me, :]).astype(bf),
            "cmat": _CMAT,
        })
    return in_maps


def kernel(**inputs) -> np.ndarray:
    with_bias = bool(np.any(np.asarray(inputs["b"]))
                     or np.any(np.asarray(inputs["gate_b"])))
    key = ("nc", with_bias)
    if key not in _cache:
        _cache[key] = _build(with_bias)
        _cache["nc"] = _cache[key]
    nc = _cache[key]
    in_maps = _make_in_maps(inputs)
    res = run_bass_kernel_spmd(nc, in_maps, core_ids=list(range(KC)))
    out = np.empty((N, MD), np.float32)
    for k in range(KC):
        yk = res.results[k]["y"]
        for ch in range(4):
            base = 0
            for spp in RS_GROUPS[ch]:
                rows = spp * 128 // KC
                g0 = ch * 512 + base * 128 + k * rows
                l0 = ch * 64 + base * 128 // KC
                out[g0:g0 + rows] = yk[l0:l0 + rows]
                base += spp
    return out
